# revision 2
# baseline (speedup 1.0000x reference)
"""Trainium2 Bass kernel for nn_CropCrossEntropy.

Reference computation (see reference.py):
    gt[i, y, x] = 1 inside the inclusive box [y0:y1, x0:x1] of image i, else 0
    loss = -(log(mp)*gt + log1p(-mp)*(1-gt)).mean()

Reformulation: with q = mp inside the box and q = 1-mp outside,
    loss = -mean(ln q),   q = sigma*(mp - 0.5) + 0.5,   sigma = 2*gt - 1.

sigma is a small-rank product of row/col box indicators, built by the
TensorEngine in PSUM from tiny host-precomputed fp8 factors. Per element the
device does ONE VectorE op u = (mp - 0.5) * sigma (scalar_tensor_tensor) and
ONE ScalarE op ln(2u + 1) = ln 2 + ln q (activation with free affine
scale/bias and fused per-partition accumulation); the host subtracts N*ln2.
The kernel is HBM-bandwidth bound (~16.8 MB/core).

Sharding: data-parallel over the fused (b*r)=512 image dim, 64 images/core on
8 cores; each core returns per-partition partial sums; the host reduces.

Layout (v2, "flat-transposed"): the whole 16 MB core slice is ONE DRAM/SBUF
tensor [128, 32768] fp32 with 128 KB contiguous per partition. Column block
[2048c, 2048(c+1)) is "chunk" c (4 images); within it partition p holds 8
consecutive rows of image 4c + p//32 (rows 8*(p%32)..+7), identical to the
old per-chunk view, so the sigma factorization is unchanged. Benefits:
  - DMA piece sizes are decoupled from compute granularity (big 2 MB pieces
    mid-stream for low overhead, small pieces at both ends for pipeline
    ramp/taper), all full-128-partition with >=2 KB lines.
  - everything is SBUF-resident; no buffer recycling can gate the DMA stream.

sigma per chunk: PSUM tile [128, 2048] = 4 banks; bank b is a K=9 matmul.
The 4 bank matmuls issue to distinct PE row-groups (tile_position=(32b, 0),
operands staged at SBUF partitions 32b..32b+8) so they execute concurrently
in the 32x32-subarray grid - ~4x less TensorE span than serial banks, which
otherwise co-bottlenecks with DMA (the PE runs cold/throttled here).

masks (fp8e4, exact for values {0, 1, 2, -1}): one [36, 10240] DRAM tensor;
4 row-group DMAs issued on the Scalar HWDGE queue so they complete
independently of the bulk mp stream on the Sync queue (a shared-queue masks
load was observed to complete ~5 us late, stalling all compute).
"""

from contextlib import ExitStack

import ml_dtypes
import numpy as np

import concourse.bass as bass
import concourse.tile as tile
from concourse import bacc, mybir
from concourse.bass_utils import run_bass_kernel_spmd

N_CORES = 8
B, R, H, W = 32, 16, 256, 256
IMGS = B * R                      # 512
IMGS_PER_CORE = IMGS // N_CORES   # 64
P = 128
CHUNK_IMGS = 4
N_CHUNKS = IMGS_PER_CORE // CHUNK_IMGS  # 16
CH = CHUNK_IMGS * H * W // P      # 2048 cols per chunk (8 image rows/partition)
TOT = N_CHUNKS * CH               # 32768 cols per core
BANK = 512
N_BANKS = CH // BANK              # 4
K = 9                             # mask rank: 4 images x 2 col-halves + const
MCOLS = N_CHUNKS * P + N_CHUNKS * BANK  # 2048 lhsT cols + 8192 rhs cols
N_ELEMS = IMGS * H * W
LN2 = float(np.log(2.0))

# mp DMA pieces (in cols): small head so compute starts early, 2 MB middle
# pieces for low per-transfer overhead, tapered tail so the last compute
# piece is small. Chunk boundaries (2048) never straddle a piece except for
# the final chunk, which is consumed in 512-col slices.
PIECES = [
    (0, 2048),
    (2048, 6144),
    (6144, 10240),
    (10240, 14336),
    (14336, 18432),
    (18432, 22528),
    (22528, 26624),
    (26624, 28672),
    (28672, 30720),
    (30720, 31744),
    (31744, 32256),
    (32256, 32768),
]
N_ACC = 12                        # 7 pair-ACTs + 1 + 4 tail ACTs

_cached_nc = None


def _build_nc():
    nc = bacc.Bacc("TRN2", target_bir_lowering=False, debug=False)

    mp = nc.dram_tensor(
        "mp", [P, TOT], mybir.dt.float32, kind="ExternalInput"
    ).ap()
    masks = nc.dram_tensor(
        "masks", [4 * K, MCOLS], mybir.dt.float8e4, kind="ExternalInput"
    ).ap()
    acc_out = nc.dram_tensor(
        "acc", [P, N_ACC], mybir.dt.float32, kind="ExternalOutput"
    ).ap()

    with tile.TileContext(nc) as tc, ExitStack() as ctx:
        mask_pool = ctx.enter_context(tc.tile_pool(name="masks", bufs=1))
        mp_pool = ctx.enter_context(tc.tile_pool(name="mp", bufs=1))
        u_pool = ctx.enter_context(tc.tile_pool(name="u", bufs=2))
        scr_pool = ctx.enter_context(tc.tile_pool(name="scr", bufs=2))
        acc_pool = ctx.enter_context(tc.tile_pool(name="acc", bufs=1))
        ps_pool = ctx.enter_context(tc.tile_pool(name="sig", bufs=2, space="PSUM"))

        masks_t = mask_pool.tile([P, MCOLS], mybir.dt.float8e4)
        mp_t = mp_pool.tile([P, TOT], mybir.dt.float32)
        acc_t = acc_pool.tile([P, N_ACC], mybir.dt.float32)

        # masks per PE row-group, on the Scalar HWDGE queue: independent of
        # the bulk stream, and bank b's matmuls gate only on DMA b.
        for b in range(N_BANKS):
            nc.scalar.dma_start(
                masks_t[32 * b : 32 * b + K, :], masks[K * b : K * (b + 1), :]
            )
        # bulk mp stream on the Sync HWDGE queue
        for lo, hi in PIECES:
            nc.sync.dma_start(mp_t[:, lo:hi], mp[:, lo:hi])

        def sigma(c):
            """sigma for chunk c: 4 concurrent K=9 row-group matmuls."""
            sg_t = ps_pool.tile([P, CH], mybir.dt.float32, tag="sg")
            for b in range(N_BANKS):
                nc.tensor.matmul(
                    sg_t[:, b * BANK : (b + 1) * BANK],
                    masks_t[32 * b : 32 * b + K, c * P : (c + 1) * P],
                    masks_t[
                        32 * b : 32 * b + K,
                        N_CHUNKS * P + c * BANK : N_CHUNKS * P + (c + 1) * BANK,
                    ],
                    start=True,
                    stop=True,
                    tile_position=(32 * b, 0),
                )
            return sg_t

        def dve(u_t, ulo, c, clo, n):
            """u[ulo:ulo+n] = (mp[chunk c cols clo:clo+n] - 0.5) * sigma"""
            nc.vector.scalar_tensor_tensor(
                u_t[:, ulo : ulo + n],
                mp_t[:, c * CH + clo : c * CH + clo + n],
                0.5,
                sg_tiles[c][:, clo : clo + n],
                mybir.AluOpType.subtract,
                mybir.AluOpType.mult,
            )

        def act(u_t, lo, n, col):
            """acc[col] += sum ln(2*u + 1) over u[lo:lo+n] (per partition)"""
            scr_t = scr_pool.tile([P, 4096], mybir.dt.bfloat16, tag="scr")
            nc.scalar.activation(
                scr_t[:, :n],
                u_t[:, lo : lo + n],
                mybir.ActivationFunctionType.Ln,
                bias=1.0,
                scale=2.0,
                accum_out=acc_t[:, col : col + 1],
            )

        sg_tiles = {}
        # chunks 0..13 in pairs: 2x DVE(2048) -> 1x ACT(4096)
        for t in range(7):
            u_t = u_pool.tile([P, 4096], mybir.dt.float32, tag="u")
            for half, c in enumerate((2 * t, 2 * t + 1)):
                sg_tiles[c] = sigma(c)
                dve(u_t, half * CH, c, 0, CH)
            act(u_t, 0, 4096, t)
        # chunk 14: DVE(2048) -> ACT(2048)
        u_t = u_pool.tile([P, 4096], mybir.dt.float32, tag="u")
        sg_tiles[14] = sigma(14)
        dve(u_t, 0, 14, 0, CH)
        act(u_t, 0, CH, 7)
        # ship the bulk of acc; only 4 columns remain at the end
        nc.sync.dma_start(acc_out[:, :8], acc_t[:, :8])
        # chunk 15 tapered: 4x (DVE(512) -> ACT(512))
        u_t = u_pool.tile([P, 4096], mybir.dt.float32, tag="u")
        sg_tiles[15] = sigma(15)
        for s in range(4):
            dve(u_t, s * BANK, 15, s * BANK, BANK)
            act(u_t, s * BANK, BANK, 8 + s)
        nc.sync.dma_start(acc_out[:, 8:], acc_t[:, 8:])

    nc.compile()
    return nc


def _get_nc():
    global _cached_nc
    if _cached_nc is None:
        _cached_nc = _build_nc()
    return _cached_nc


def _make_in_maps(mask_pred, pos_gt):
    mp = np.ascontiguousarray(np.asarray(mask_pred), dtype=np.float32).reshape(
        IMGS, H * W
    )
    pg = np.asarray(pos_gt).reshape(IMGS, 4).astype(np.int64)
    rows = np.arange(H)[None, :]
    cols = np.arange(W)[None, :]
    y0, x0, y1, x1 = (pg[:, k][:, None] for k in range(4))
    rowind = ((rows >= y0) & (rows <= y1)).astype(np.float32)  # (512, 256)
    colind = ((cols >= x0) & (cols <= x1)).astype(np.float32)  # (512, 256)

    # lhsT row for bank b: image row 8*(p%32) + 2b + h, p in [32i, 32i+32)
    q32 = np.arange(32)
    bank_rows = 8 * q32[None, :] + 2 * np.arange(N_BANKS)[:, None]  # (4, 32)

    in_maps = []
    for cid in range(N_CORES):
        sl = slice(cid * IMGS_PER_CORE, (cid + 1) * IMGS_PER_CORE)
        # [chunk, p, j] -> [p, chunk*2048 + j]: 128 KB contiguous/partition
        mp_c = np.ascontiguousarray(
            mp[sl].reshape(N_CHUNKS, P, CH).transpose(1, 0, 2)
        ).reshape(P, TOT)
        rc = rowind[sl].reshape(N_CHUNKS, CHUNK_IMGS, H)
        cc = colind[sl].reshape(N_CHUNKS, CHUNK_IMGS, W)

        lhs = np.zeros((N_CHUNKS, N_BANKS, K, P), np.float32)
        rhs = np.zeros((N_CHUNKS, K, BANK), np.float32)
        for i in range(CHUNK_IMGS):
            for h in range(2):
                lhs[:, :, 2 * i + h, 32 * i : 32 * (i + 1)] = rc[:, i][
                    :, bank_rows + h
                ]
                rhs[:, 2 * i + h, 256 * h : 256 * (h + 1)] = 2.0 * cc[:, i]
        lhs[:, :, 8, :] = 1.0
        rhs[:, 8, :] = -1.0

        # masks row-group b (rows 9b..9b+9): [lhsT for 16 chunks | rhs copy]
        mk = np.empty((N_BANKS, K, MCOLS), np.float32)
        rhs_flat = np.ascontiguousarray(rhs.transpose(1, 0, 2)).reshape(K, -1)
        for b in range(N_BANKS):
            mk[b, :, : N_CHUNKS * P] = np.ascontiguousarray(
                lhs[:, b].transpose(1, 0, 2)
            ).reshape(K, -1)
            mk[b, :, N_CHUNKS * P :] = rhs_flat
        in_maps.append(
            {
                "mp": mp_c,
                "masks": mk.reshape(4 * K, MCOLS).astype(
                    ml_dtypes.float8_e4m3fn
                ),
            }
        )
    return in_maps


def _run(mask_pred, pos_gt, trace=False, **run_kwargs):
    nc = _get_nc()
    in_maps = _make_in_maps(mask_pred, pos_gt)
    res = run_bass_kernel_spmd(
        nc, in_maps, core_ids=list(range(N_CORES)), trace=trace, **run_kwargs
    )
    total = 0.0
    for r in res.results:
        total += float(np.sum(np.asarray(r["acc"], dtype=np.float64)))
    # acc sums ln(2u+1) = ln2 + ln(q): subtract the known N*ln2 shift
    loss = np.float32(-((total - N_ELEMS * LN2) / N_ELEMS))
    return loss, res


def kernel(mask_pred, pos_gt):
    loss, _ = _run(mask_pred, pos_gt, trace=False)
    return loss
